# revision 29
# baseline (speedup 1.0000x reference)
"""Trainium2 Bass kernel for nn_Attention_8924942041930 (sparse_attention).

Reference computation (per batch of N=384 tokens = [t(64) | it(64) | s(256)]):
  qkv = x @ w_qkv
  mix attention: (t+s) queries over all N keys
  inherent attention: it queries over it keys only
  out = concat(t, it, s) @ w_proj + b_proj

Sharding: data-parallel over batch, 8 batches per NeuronCore (64 total / 8).

Per-core dataflow (all matmuls bf16 with f32 PSUM accumulation):
  - host supplies x^T [768, 3072] bf16 so the f-contraction lands on SBUF
    partitions with no on-chip transposes
  - qk^T = w_qkv^T @ x^T per batch -> [hd, token] per head (head pair per
    128-partition chunk: even head rows 0-63, odd head rows 64-127)
  - v = x @ w_v -> [token, hd] per head (PV lhsT needs token on partitions)
  - S^T[k, q] = k^T.T @ q^T per (head, k-chunk); K=64 so head pairs are
    row-packed in the PE array via tile_position rows 0/64 (MMs emitted
    A/B-interleaved so they run concurrently)
  - attn^T = exp(S^T * scale) on ScalarE (logits are tiny: no max-subtract)
  - ts^T = v.T @ attn^T col-packed per head pair (tile_position cols 0/64)
    so the pair lands stacked [128, q] = the proj rhs chunk layout
  - denominators = ones.T @ attn^T as M=1 matmuls col-tiled 4 heads/bank
    (emitted 4-head-interleaved for concurrency); inherent sums share the
    same bank at cols N:N+64
  - normalize: fast reciprocal + DRAM-bounce partition-broadcast + DVE mult
  - inherent attention reuses S^T chunk 0 rows/cols 64:128 (it x it block)
  - out^T = w_proj^T @ attnout^T + b_proj -> host transposes back

Scheduling: the PE instruction queue is strict FIFO, and the attention
matmuls are gated by ScalarE exp throughput, so qk/v chains of batch b+1
and proj chains of batch b-1 are emitted interleaved between the attention
groups of batch b as PE filler work.
"""

import sys

import numpy as np

if "/opt/trn_rl_repo" not in sys.path:
    sys.path.insert(0, "/opt/trn_rl_repo")

import ml_dtypes

B = 64
N = 384
DIM = 768
H = 12
HD = 64
T_SIZE = 64
S_SIZE = 256
SCALE = HD ** -0.5
NCORES = 8
BPC = B // NCORES  # batches per core
FCH = DIM // 128  # feature chunks of 128
P = 128
NW = N + T_SIZE  # sums/bcast width: mix cols 0:N, inherent cols N:N+64

BF16 = ml_dtypes.bfloat16


def build(n_batches=BPC, sim_safe=False):
    """Build the per-core Bass program (SPMD: same NEFF, per-core data)."""
    import concourse.mybir as mybir
    import concourse.tile as tile
    from concourse import bacc
    from collections import deque
    from contextlib import ExitStack

    bf16 = mybir.dt.bfloat16
    f32 = mybir.dt.float32
    Exp = mybir.ActivationFunctionType.Exp
    mult = mybir.AluOpType.mult
    ntok = n_batches * N

    nc = bacc.Bacc("TRN2", target_bir_lowering=False, debug=False,
                   num_devices=NCORES)
    xT = nc.dram_tensor("xT", [DIM, ntok], bf16, kind="ExternalInput")
    wqkv = nc.dram_tensor("wqkv", [DIM, 3 * DIM], bf16, kind="ExternalInput")
    wproj = nc.dram_tensor("wproj", [DIM, DIM], bf16, kind="ExternalInput")
    bproj = nc.dram_tensor("bproj", [DIM], f32, kind="ExternalInput")
    outT = nc.dram_tensor("outT", [DIM, ntok], f32, kind="ExternalOutput")

    xT_r = xT.rearrange("(o p) t -> p o t", p=P)
    wqkv_r = wqkv.rearrange("(o p) c -> p o c", p=P)
    wproj_r = wproj.rearrange("(o p) c -> p o c", p=P)
    bproj_r = bproj.rearrange("(o p) -> p o", p=P)
    outT_r = outT.rearrange("(o p) t -> p o t", p=P)

    with tile.TileContext(nc) as tc, ExitStack() as ctx:
        const = ctx.enter_context(tc.tile_pool(name="const", bufs=1))
        qk_pool = ctx.enter_context(tc.tile_pool(name="qk", bufs=2))
        v_pool = ctx.enter_context(tc.tile_pool(name="v", bufs=2))
        attn_pool = ctx.enter_context(tc.tile_pool(name="attn", bufs=12))
        ao_pool = ctx.enter_context(tc.tile_pool(name="ao", bufs=2))
        rec_pool = ctx.enter_context(tc.tile_pool(name="rec", bufs=3))
        bc_pool = ctx.enter_context(tc.tile_pool(name="bc", bufs=3))
        out_pool = ctx.enter_context(tc.tile_pool(name="outp", bufs=2))
        dram_pool = ctx.enter_context(tc.tile_pool(name="dramp", bufs=3,
                                                   space="DRAM"))
        # PSUM budget: 8 banks = work(5, shared qkv/proj/S/sums) + pv(3)
        work_ps = ctx.enter_context(tc.tile_pool(name="workps", bufs=4,
                                                 space="PSUM"))
        pv_ps = ctx.enter_context(tc.tile_pool(name="pvps", bufs=4,
                                               space="PSUM"))

        # --- persistent tensors (per-chunk tiles => finer DMA deps) ---
        xT_ch = [const.tile([P, ntok], bf16, tag=f"xT{o}", name=f"xT{o}")
                 for o in range(FCH)]
        wqkv_ch = [const.tile([P, 3 * DIM], bf16, tag=f"wqkv{o}",
                              name=f"wqkv{o}") for o in range(FCH)]
        wproj_ch = [const.tile([P, DIM], bf16, tag=f"wproj{o}",
                               name=f"wproj{o}") for o in range(FCH)]
        # phase 1: slices the first chains touch, alternating queues
        for o in range(FCH):
            eng = nc.sync if o % 2 == 0 else nc.gpsimd
            eng.dma_start(wqkv_ch[o][:, 0:N], wqkv_r[:, o, 0:N])
            eng.dma_start(xT_ch[o][:, 0:N], xT_r[:, o, 0:N])
        # phase 2: rest of wqkv (later qk chunks + v weights)
        for o in range(FCH):
            eng = nc.sync if o % 2 == 0 else nc.gpsimd
            eng.dma_start(wqkv_ch[o][:, N:2 * DIM], wqkv_r[:, o, N:2 * DIM])
            eng.dma_start(wqkv_ch[o][:, 2 * DIM:3 * DIM],
                          wqkv_r[:, o, 2 * DIM:3 * DIM])
        # phase 3: remaining activations and proj weights
        q = (ntok - N) // 3 if n_batches > 1 else 0
        for piece in range(3 if q else 0):
            for o in range(FCH):
                eng = nc.sync if o % 2 == 0 else nc.gpsimd
                eng.dma_start(xT_ch[o][:, N + piece * q:N + (piece + 1) * q],
                              xT_r[:, o, N + piece * q:N + (piece + 1) * q])
        for o in range(FCH):
            nc.sync.dma_start(wproj_ch[o][:], wproj_r[:, o, :])
        bproj_sb = const.tile([P, FCH], f32, tag="bproj")
        nc.sync.dma_start(bproj_sb[:], bproj_r[:])
        ones_sb = const.tile([P, 1], bf16, tag="ones")
        nc.gpsimd.memset(ones_sb[:], 1.0)

        qkTs = {}   # b -> qkT tile
        v_sbs = {}  # b -> v tile
        aos = {}    # b -> attnoutT tile

        def qkv_chains(b):
            """Yield one emitter per accumulation chain (18 per batch)."""
            t0 = b * N
            qkT = qk_pool.tile([P, 2 * FCH, N], bf16, tag="qkT")
            qkTs[b] = qkT

            def qk_chain(cc):
                def emit():
                    ps = work_ps.tile([P, N], mybir.dt.float32, tag="w")
                    for fo in range(FCH):
                        nc.tensor.matmul(
                            ps[:],
                            wqkv_ch[fo][:, cc * P:(cc + 1) * P],
                            xT_ch[fo][:, t0:t0 + N],
                            start=(fo == 0), stop=(fo == FCH - 1),
                        )
                    if cc % 3 == 2:
                        nc.scalar.copy(qkT[:, cc, :], ps[:])
                    else:
                        nc.vector.tensor_copy(qkT[:, cc, :], ps[:])
                return emit

            for cc in range(2 * FCH):
                yield qk_chain(cc)

            v_sb = v_pool.tile([P, 3, H, HD], bf16, tag="v")
            v_sbs[b] = v_sb

            def v_chain(tch, half):
                def emit():
                    ps = work_ps.tile([P, N], mybir.dt.float32, tag="w")
                    for fo in range(FCH):
                        nc.tensor.matmul(
                            ps[:],
                            xT_ch[fo][:, t0 + tch * P:t0 + (tch + 1) * P],
                            wqkv_ch[fo][:,
                                         2 * DIM + half * N:
                                         2 * DIM + (half + 1) * N],
                            start=(fo == 0), stop=(fo == FCH - 1),
                        )
                    nc.vector.tensor_copy(
                        v_sb[:, tch, half * 6:(half + 1) * 6, :],
                        ps[:].rearrange("p (h d) -> p h d", d=HD),
                    )
                return emit

            for tch in range(3):
                for half in range(2):
                    yield v_chain(tch, half)

        def proj_chains(b):
            """Yield one emitter per proj output chunk (6) + the store."""
            t0 = b * N
            ao = aos.pop(b)
            outstage = out_pool.tile([P, FCH, N], mybir.dt.float32,
                                     tag="outs")

            def proj_chain(cc):
                def emit():
                    ps = work_ps.tile([P, N], mybir.dt.float32, tag="w")
                    for fo in range(FCH):
                        nc.tensor.matmul(
                            ps[:],
                            wproj_ch[fo][:, cc * P:(cc + 1) * P],
                            ao[:, fo, :],
                            start=(fo == 0), stop=(fo == FCH - 1),
                        )
                    nc.vector.tensor_scalar_add(outstage[:, cc, :], ps[:],
                                                bproj_sb[:, cc:cc + 1])
                    eng = nc.sync if cc % 2 == 0 else nc.gpsimd
                    eng.dma_start(outT_r[:, cc, t0:t0 + N],
                                  outstage[:, cc, :])
                return emit

            for cc in range(FCH):
                yield proj_chain(cc)

        def emit_attention(b, fillers):
            """Attention for batch b; pops PE filler chains at stall points."""
            qkT = qkTs.pop(b)
            v_sb = v_sbs[b]

            def fill(k):
                for _ in range(k):
                    if fillers:
                        fillers.popleft()()

            attnoutT = ao_pool.tile([P, FCH, N], bf16, tag="aoT")
            aos[b] = attnoutT

            for g in range(3):  # head groups of 4 (two pairs)
                heads = [4 * g + i for i in range(4)]
                attnTs = {}
                for h in heads:
                    attnTs[h] = attn_pool.tile([P, 3, N], bf16, tag="attnT",
                                               name=f"attnT_{h}")
                # S^T + exp: pairs row-packed; A/B interleaved per kc so the
                # two K=64 matmuls run concurrently in the array
                for pair in range(2):
                    hA, hB = heads[2 * pair], heads[2 * pair + 1]
                    j = hA // 2
                    for kc in range(3):
                        sA = work_ps.tile([P, N], mybir.dt.float32, tag="w", name="sA")
                        sB = work_ps.tile([P, N], mybir.dt.float32, tag="w", name="sB")
                        nc.tensor.matmul(
                            sA[:], qkT[0:64, FCH + j, kc * P:(kc + 1) * P],
                            qkT[0:64, j, :], start=True, stop=True,
                            tile_position=(0, 0))
                        nc.tensor.matmul(
                            sB[:], qkT[64:128, FCH + j, kc * P:(kc + 1) * P],
                            qkT[64:128, j, :], start=True, stop=True,
                            tile_position=(64, 0))
                        nc.scalar.activation(attnTs[hA][:, kc, :], sA[:],
                                             Exp, scale=SCALE)
                        nc.scalar.activation(attnTs[hB][:, kc, :], sB[:],
                                             Exp, scale=SCALE)
                        # cover the exp-paced s-slot recycle with dense work
                        fill(1)

                # denominators: 4 heads col-tiled; idx-inner emission so the
                # four M=1 matmuls stream concurrently
                sums = work_ps.tile([P, NW], mybir.dt.float32, tag="w",
                                    name="sums")
                if sim_safe:
                    nc.vector.memset(sums[:], 1.0)
                for kc in range(3):
                    for idx, h in enumerate(heads):
                        nc.tensor.matmul(
                            sums[32 * idx:32 * idx + 1, 0:N],
                            ones_sb[:, 0:1], attnTs[h][:, kc, :],
                            start=(kc == 0), stop=(kc == 2),
                            tile_position=(0, 32 * idx),
                            skip_group_check=(idx > 0),
                        )
                for idx, h in enumerate(heads):
                    nc.tensor.matmul(
                        sums[32 * idx:32 * idx + 1, N:NW],
                        ones_sb[64:128, 0:1], attnTs[h][64:128, 0, 64:128],
                        start=True, stop=True,
                        tile_position=(64, 32 * idx),
                        skip_group_check=(idx > 0),
                    )
                recips = rec_pool.tile([P, NW], mybir.dt.float32, tag="rec")
                nc.vector.reciprocal_approx_fast(recips[0:97, :],
                                                 sums[0:97, :])
                rec_dram = dram_pool.tile([4, NW], mybir.dt.float32, tag="rd")
                nc.gpsimd.dma_start(
                    rec_dram[:],
                    recips[:].rearrange("(a b) n -> a b n", b=32)[:, 0, :],
                )
                # broadcast: [128, pair, NW]; rows 0-63 = even (A) recips,
                # rows 64-127 = odd (B) recips
                bcast_g = bc_pool.tile([P, 2, NW], mybir.dt.float32,
                                       tag="bc")
                rec_pairs = rec_dram[:].rearrange("(p two) n -> p two n",
                                                  two=2)
                nc.gpsimd.dma_start(
                    bcast_g[0:64, :, :],
                    rec_pairs[:, 0, :].partition_broadcast(64))
                nc.gpsimd.dma_start(
                    bcast_g[64:128, :, :],
                    rec_pairs[:, 1, :].partition_broadcast(64))
                fill(1)

                for pair in range(2):
                    hA, hB = heads[2 * pair], heads[2 * pair + 1]
                    jc = hA // 2
                    attnA, attnB = attnTs[hA], attnTs[hB]

                    # PV mix: pair col-packed -> [128, N] stacked ts^T
                    pv = pv_ps.tile([P, N], mybir.dt.float32, tag="pv")
                    for kc in range(3):
                        nc.tensor.matmul(
                            pv[0:64, :], v_sb[:, kc, hA, :], attnA[:, kc, :],
                            start=(kc == 0), stop=(kc == 2),
                            tile_position=(0, 0),
                        )
                        nc.tensor.matmul(
                            pv[64:128, :], v_sb[:, kc, hB, :],
                            attnB[:, kc, :],
                            start=(kc == 0), stop=(kc == 2),
                            tile_position=(0, 64),
                            skip_group_check=True,
                        )
                    # PV inherent: it queries over it keys (chunk 0 64:128)
                    pvit = pv_ps.tile([P, T_SIZE], mybir.dt.float32,
                                      tag="pv")
                    nc.tensor.matmul(
                        pvit[0:64, :], v_sb[64:128, 0, hA, :],
                        attnA[64:128, 0, 64:128],
                        start=True, stop=True, tile_position=(64, 0),
                    )
                    nc.tensor.matmul(
                        pvit[64:128, :], v_sb[64:128, 0, hB, :],
                        attnB[64:128, 0, 64:128],
                        start=True, stop=True, tile_position=(64, 64),
                        skip_group_check=True,
                    )
                    nc.vector.tensor_tensor(
                        attnoutT[:, jc, 0:T_SIZE], pv[:, 0:T_SIZE],
                        bcast_g[:, pair, 0:T_SIZE], mult)
                    nc.vector.tensor_tensor(
                        attnoutT[:, jc, 2 * T_SIZE:N], pv[:, 2 * T_SIZE:N],
                        bcast_g[:, pair, 2 * T_SIZE:N], mult)
                    nc.vector.tensor_tensor(
                        attnoutT[:, jc, T_SIZE:2 * T_SIZE], pvit[:],
                        bcast_g[:, pair, N:NW], mult)
                    if pair == 0:
                        fill(1)

        # prologue: batch 0's qk/v chains run first
        for emit in qkv_chains(0):
            emit()
        for b in range(n_batches):
            fillers = deque()
            if b + 1 < n_batches:
                fillers.extend(qkv_chains(b + 1))
            if b >= 1:
                fillers.extend(proj_chains(b - 1))
            emit_attention(b, fillers)
            while fillers:
                fillers.popleft()()
        for emit in proj_chains(n_batches - 1):
            emit()

    nc.compile()
    return nc


_CACHED_NC = None


def _get_nc():
    global _CACHED_NC
    if _CACHED_NC is None:
        _CACHED_NC = build(BPC)
    return _CACHED_NC


def kernel(x, w_qkv, w_proj, b_proj):
    from concourse.bass_utils import run_bass_kernel_spmd

    nc = _get_nc()

    wqkv_bf = np.ascontiguousarray(w_qkv.astype(BF16))
    wproj_bf = np.ascontiguousarray(w_proj.astype(BF16))
    bproj_f = np.ascontiguousarray(b_proj.astype(np.float32))

    in_maps = []
    for c in range(NCORES):
        xc = x[c * BPC:(c + 1) * BPC].reshape(BPC * N, DIM)
        xT = np.ascontiguousarray(xc.T.astype(BF16))
        in_maps.append({
            "xT": xT,
            "wqkv": wqkv_bf,
            "wproj": wproj_bf,
            "bproj": bproj_f,
        })

    res = run_bass_kernel_spmd(nc, in_maps, core_ids=list(range(NCORES)))
    outs = [
        np.ascontiguousarray(res.results[c]["outT"].T).reshape(BPC, N, DIM)
        for c in range(NCORES)
    ]
    return np.concatenate(outs, axis=0)


if __name__ == "__main__":
    rng = np.random.default_rng(0)
    x = rng.standard_normal((B, N, DIM), dtype=np.float32)
    w_qkv = (rng.standard_normal((DIM, 3 * DIM), dtype=np.float32) * 0.02)
    w_proj = (rng.standard_normal((DIM, DIM), dtype=np.float32) * 0.02)
    b_proj = np.zeros((DIM,), dtype=np.float32)
    out = kernel(x, w_qkv, w_proj, b_proj)
    print("out", out.shape, out.dtype, float(np.abs(out).max()))


# revision 30
# speedup vs baseline: 1.0895x; 1.0895x over previous
"""Trainium2 Bass kernel for nn_Attention_8924942041930 (sparse_attention).

Reference computation (per batch of N=384 tokens = [t(64) | it(64) | s(256)]):
  qkv = x @ w_qkv
  mix attention: (t+s) queries over all N keys
  inherent attention: it queries over it keys only
  out = concat(t, it, s) @ w_proj + b_proj

Sharding: data-parallel over batch, 8 batches per NeuronCore (64 total / 8).

Per-core dataflow (all matmuls bf16 with f32 PSUM accumulation):
  - host supplies x^T [768, 3072] bf16 so the f-contraction lands on SBUF
    partitions with no on-chip transposes
  - qk^T = w_qkv^T @ x^T per batch -> [hd, token] per head (head pair per
    128-partition chunk: even head rows 0-63, odd head rows 64-127)
  - v = x @ w_v -> [token, hd] per head (PV lhsT needs token on partitions)
  - S^T[k, q] = k^T.T @ q^T per (head, k-chunk); K=64 so head pairs are
    row-packed in the PE array via tile_position rows 0/64 (MMs emitted
    A/B-interleaved so they run concurrently)
  - attn^T = exp(S^T * scale) on ScalarE (logits are tiny: no max-subtract)
  - ts^T = v.T @ attn^T col-packed per head pair (tile_position cols 0/64)
    so the pair lands stacked [128, q] = the proj rhs chunk layout
  - denominators = ones.T @ attn^T as M=1 matmuls col-tiled 4 heads/bank
    (emitted 4-head-interleaved for concurrency); inherent sums share the
    same bank at cols N:N+64
  - normalize: fast reciprocal + DRAM-bounce partition-broadcast + DVE mult
  - inherent attention reuses S^T chunk 0 rows/cols 64:128 (it x it block)
  - out^T = w_proj^T @ attnout^T + b_proj -> host transposes back

Scheduling: the PE instruction queue is strict FIFO, and the attention
matmuls are gated by ScalarE exp throughput, so qk/v chains of batch b+1
and proj chains of batch b-1 are emitted interleaved between the attention
groups of batch b as PE filler work.
"""

import sys

import numpy as np

if "/opt/trn_rl_repo" not in sys.path:
    sys.path.insert(0, "/opt/trn_rl_repo")

import ml_dtypes

B = 64
N = 384
DIM = 768
H = 12
HD = 64
T_SIZE = 64
S_SIZE = 256
SCALE = HD ** -0.5
NCORES = 8
BPC = B // NCORES  # batches per core
FCH = DIM // 128  # feature chunks of 128
P = 128
NW = N + T_SIZE  # sums/bcast width: mix cols 0:N, inherent cols N:N+64

BF16 = ml_dtypes.bfloat16


def build(n_batches=BPC, sim_safe=False):
    """Build the per-core Bass program (SPMD: same NEFF, per-core data)."""
    import concourse.mybir as mybir
    import concourse.tile as tile
    from concourse import bacc
    from collections import deque
    from contextlib import ExitStack

    bf16 = mybir.dt.bfloat16
    f32 = mybir.dt.float32
    Exp = mybir.ActivationFunctionType.Exp
    mult = mybir.AluOpType.mult
    ntok = n_batches * N

    nc = bacc.Bacc("TRN2", target_bir_lowering=False, debug=False,
                   num_devices=NCORES)
    xT = nc.dram_tensor("xT", [DIM, ntok], bf16, kind="ExternalInput")
    wqkv = nc.dram_tensor("wqkv", [DIM, 3 * DIM], bf16, kind="ExternalInput")
    wproj = nc.dram_tensor("wproj", [DIM, DIM], bf16, kind="ExternalInput")
    bproj = nc.dram_tensor("bproj", [DIM], f32, kind="ExternalInput")
    outT = nc.dram_tensor("outT", [DIM, ntok], f32, kind="ExternalOutput")

    xT_r = xT.rearrange("(o p) t -> p o t", p=P)
    wqkv_r = wqkv.rearrange("(o p) c -> p o c", p=P)
    wproj_r = wproj.rearrange("(o p) c -> p o c", p=P)
    bproj_r = bproj.rearrange("(o p) -> p o", p=P)
    outT_r = outT.rearrange("(o p) t -> p o t", p=P)

    with tile.TileContext(nc) as tc, ExitStack() as ctx:
        const = ctx.enter_context(tc.tile_pool(name="const", bufs=1))
        qk_pool = ctx.enter_context(tc.tile_pool(name="qk", bufs=2))
        v_pool = ctx.enter_context(tc.tile_pool(name="v", bufs=2))
        attn_pool = ctx.enter_context(tc.tile_pool(name="attn", bufs=8))
        ao_pool = ctx.enter_context(tc.tile_pool(name="ao", bufs=2))
        rec_pool = ctx.enter_context(tc.tile_pool(name="rec", bufs=3))
        bc_pool = ctx.enter_context(tc.tile_pool(name="bc", bufs=3))
        out_pool = ctx.enter_context(tc.tile_pool(name="outp", bufs=2))
        dram_pool = ctx.enter_context(tc.tile_pool(name="dramp", bufs=3,
                                                   space="DRAM"))
        # PSUM budget: 8 banks = work(5, shared qkv/proj/S/sums) + pv(3)
        work_ps = ctx.enter_context(tc.tile_pool(name="workps", bufs=4,
                                                 space="PSUM"))
        pv_ps = ctx.enter_context(tc.tile_pool(name="pvps", bufs=4,
                                               space="PSUM"))

        # --- persistent tensors (per-chunk tiles => finer DMA deps) ---
        xT_ch = [const.tile([P, ntok], bf16, tag=f"xT{o}", name=f"xT{o}")
                 for o in range(FCH)]
        wqkv_ch = [const.tile([P, 3 * DIM], bf16, tag=f"wqkv{o}",
                              name=f"wqkv{o}") for o in range(FCH)]
        wproj_ch = [const.tile([P, DIM], bf16, tag=f"wproj{o}",
                               name=f"wproj{o}") for o in range(FCH)]
        # phase 1: slices the first chains touch, alternating queues
        for o in range(FCH):
            eng = nc.sync if o % 2 == 0 else nc.gpsimd
            eng.dma_start(wqkv_ch[o][:, 0:N], wqkv_r[:, o, 0:N])
            eng.dma_start(xT_ch[o][:, 0:N], xT_r[:, o, 0:N])
        # phase 2: rest of wqkv (later qk chunks + v weights)
        for o in range(FCH):
            eng = nc.sync if o % 2 == 0 else nc.gpsimd
            eng.dma_start(wqkv_ch[o][:, N:2 * DIM], wqkv_r[:, o, N:2 * DIM])
            eng.dma_start(wqkv_ch[o][:, 2 * DIM:3 * DIM],
                          wqkv_r[:, o, 2 * DIM:3 * DIM])
        # phase 3: remaining activations and proj weights
        q = (ntok - N) // 3 if n_batches > 1 else 0
        for piece in range(3 if q else 0):
            for o in range(FCH):
                eng = nc.sync if o % 2 == 0 else nc.gpsimd
                eng.dma_start(xT_ch[o][:, N + piece * q:N + (piece + 1) * q],
                              xT_r[:, o, N + piece * q:N + (piece + 1) * q])
        for o in range(FCH):
            nc.sync.dma_start(wproj_ch[o][:], wproj_r[:, o, :])
        bproj_sb = const.tile([P, FCH], f32, tag="bproj")
        nc.sync.dma_start(bproj_sb[:], bproj_r[:])
        ones_sb = const.tile([P, 1], bf16, tag="ones")
        nc.gpsimd.memset(ones_sb[:], 1.0)

        qkTs = {}   # b -> qkT tile
        v_sbs = {}  # b -> v tile
        aos = {}    # b -> attnoutT tile

        def qkv_chains(b):
            """Yield one emitter per accumulation chain (18 per batch)."""
            t0 = b * N
            qkT = qk_pool.tile([P, 2 * FCH, N], bf16, tag="qkT")
            qkTs[b] = qkT

            def qk_chain(cc):
                def emit():
                    ps = work_ps.tile([P, N], mybir.dt.float32, tag="w")
                    for fo in range(FCH):
                        nc.tensor.matmul(
                            ps[:],
                            wqkv_ch[fo][:, cc * P:(cc + 1) * P],
                            xT_ch[fo][:, t0:t0 + N],
                            start=(fo == 0), stop=(fo == FCH - 1),
                        )
                    if cc % 3 == 2:
                        nc.scalar.copy(qkT[:, cc, :], ps[:])
                    else:
                        nc.vector.tensor_copy(qkT[:, cc, :], ps[:])
                return emit

            for cc in range(2 * FCH):
                yield qk_chain(cc)

            v_sb = v_pool.tile([P, 3, H, HD], bf16, tag="v")
            v_sbs[b] = v_sb

            def v_chain(tch, half):
                def emit():
                    ps = work_ps.tile([P, N], mybir.dt.float32, tag="w")
                    for fo in range(FCH):
                        nc.tensor.matmul(
                            ps[:],
                            xT_ch[fo][:, t0 + tch * P:t0 + (tch + 1) * P],
                            wqkv_ch[fo][:,
                                         2 * DIM + half * N:
                                         2 * DIM + (half + 1) * N],
                            start=(fo == 0), stop=(fo == FCH - 1),
                        )
                    nc.vector.tensor_copy(
                        v_sb[:, tch, half * 6:(half + 1) * 6, :],
                        ps[:].rearrange("p (h d) -> p h d", d=HD),
                    )
                return emit

            for tch in range(3):
                for half in range(2):
                    yield v_chain(tch, half)

        def proj_chains(b):
            """Yield one emitter per proj output chunk (6) + the store."""
            t0 = b * N
            ao = aos.pop(b)
            outstage = out_pool.tile([P, FCH, N], mybir.dt.float32,
                                     tag="outs")

            def proj_chain(cc):
                def emit():
                    ps = work_ps.tile([P, N], mybir.dt.float32, tag="w")
                    for fo in range(FCH):
                        nc.tensor.matmul(
                            ps[:],
                            wproj_ch[fo][:, cc * P:(cc + 1) * P],
                            ao[:, fo, :],
                            start=(fo == 0), stop=(fo == FCH - 1),
                        )
                    nc.vector.tensor_scalar_add(outstage[:, cc, :], ps[:],
                                                bproj_sb[:, cc:cc + 1])
                    nc.sync.dma_start(outT_r[:, cc, t0:t0 + N],
                                      outstage[:, cc, :])
                return emit

            for cc in range(FCH):
                yield proj_chain(cc)

        def emit_attention(b, fillers):
            """Attention for batch b; pops PE filler chains at stall points."""
            qkT = qkTs.pop(b)
            v_sb = v_sbs[b]

            def fill(k):
                for _ in range(k):
                    if fillers:
                        fillers.popleft()()

            attnoutT = ao_pool.tile([P, FCH, N], bf16, tag="aoT")
            aos[b] = attnoutT

            for g in range(3):  # head groups of 4 (two pairs)
                heads = [4 * g + i for i in range(4)]
                attnTs = {}
                for h in heads:
                    attnTs[h] = attn_pool.tile([P, 3, N], bf16, tag="attnT",
                                               name=f"attnT_{h}")
                # S^T + exp: pairs row-packed; A/B interleaved per kc so the
                # two K=64 matmuls run concurrently in the array
                for pair in range(2):
                    hA, hB = heads[2 * pair], heads[2 * pair + 1]
                    j = hA // 2
                    for kc in range(3):
                        sA = work_ps.tile([P, N], mybir.dt.float32, tag="w", name="sA")
                        sB = work_ps.tile([P, N], mybir.dt.float32, tag="w", name="sB")
                        nc.tensor.matmul(
                            sA[:], qkT[0:64, FCH + j, kc * P:(kc + 1) * P],
                            qkT[0:64, j, :], start=True, stop=True,
                            tile_position=(0, 0))
                        nc.tensor.matmul(
                            sB[:], qkT[64:128, FCH + j, kc * P:(kc + 1) * P],
                            qkT[64:128, j, :], start=True, stop=True,
                            tile_position=(64, 0))
                        nc.scalar.activation(attnTs[hA][:, kc, :], sA[:],
                                             Exp, scale=SCALE)
                        nc.scalar.activation(attnTs[hB][:, kc, :], sB[:],
                                             Exp, scale=SCALE)
                        # cover the exp-paced s-slot recycle with dense work
                        fill(1)

                # denominators: 4 heads col-tiled; idx-inner emission so the
                # four M=1 matmuls stream concurrently
                sums = work_ps.tile([P, NW], mybir.dt.float32, tag="w",
                                    name="sums")
                if sim_safe:
                    nc.vector.memset(sums[:], 1.0)
                for kc in range(3):
                    for idx, h in enumerate(heads):
                        nc.tensor.matmul(
                            sums[32 * idx:32 * idx + 1, 0:N],
                            ones_sb[:, 0:1], attnTs[h][:, kc, :],
                            start=(kc == 0), stop=(kc == 2),
                            tile_position=(0, 32 * idx),
                            skip_group_check=(idx > 0),
                        )
                for idx, h in enumerate(heads):
                    nc.tensor.matmul(
                        sums[32 * idx:32 * idx + 1, N:NW],
                        ones_sb[64:128, 0:1], attnTs[h][64:128, 0, 64:128],
                        start=True, stop=True,
                        tile_position=(64, 32 * idx),
                        skip_group_check=(idx > 0),
                    )
                recips = rec_pool.tile([P, NW], mybir.dt.float32, tag="rec")
                nc.vector.reciprocal_approx_fast(recips[0:97, :],
                                                 sums[0:97, :])
                rec_dram = dram_pool.tile([4, NW], mybir.dt.float32, tag="rd")
                nc.gpsimd.dma_start(
                    rec_dram[:],
                    recips[:].rearrange("(a b) n -> a b n", b=32)[:, 0, :],
                )
                # broadcast: [128, pair, NW]; rows 0-63 = even (A) recips,
                # rows 64-127 = odd (B) recips
                bcast_g = bc_pool.tile([P, 2, NW], mybir.dt.float32,
                                       tag="bc")
                rec_pairs = rec_dram[:].rearrange("(p two) n -> p two n",
                                                  two=2)
                nc.gpsimd.dma_start(
                    bcast_g[0:64, :, :],
                    rec_pairs[:, 0, :].partition_broadcast(64))
                nc.gpsimd.dma_start(
                    bcast_g[64:128, :, :],
                    rec_pairs[:, 1, :].partition_broadcast(64))
                fill(1)

                for pair in range(2):
                    hA, hB = heads[2 * pair], heads[2 * pair + 1]
                    jc = hA // 2
                    attnA, attnB = attnTs[hA], attnTs[hB]

                    # PV mix: pair col-packed -> [128, N] stacked ts^T
                    pv = pv_ps.tile([P, N], mybir.dt.float32, tag="pv")
                    for kc in range(3):
                        nc.tensor.matmul(
                            pv[0:64, :], v_sb[:, kc, hA, :], attnA[:, kc, :],
                            start=(kc == 0), stop=(kc == 2),
                            tile_position=(0, 0),
                        )
                        nc.tensor.matmul(
                            pv[64:128, :], v_sb[:, kc, hB, :],
                            attnB[:, kc, :],
                            start=(kc == 0), stop=(kc == 2),
                            tile_position=(0, 64),
                            skip_group_check=True,
                        )
                    # PV inherent: it queries over it keys (chunk 0 64:128)
                    pvit = pv_ps.tile([P, T_SIZE], mybir.dt.float32,
                                      tag="pv")
                    nc.tensor.matmul(
                        pvit[0:64, :], v_sb[64:128, 0, hA, :],
                        attnA[64:128, 0, 64:128],
                        start=True, stop=True, tile_position=(64, 0),
                    )
                    nc.tensor.matmul(
                        pvit[64:128, :], v_sb[64:128, 0, hB, :],
                        attnB[64:128, 0, 64:128],
                        start=True, stop=True, tile_position=(64, 64),
                        skip_group_check=True,
                    )
                    nc.vector.tensor_tensor(
                        attnoutT[:, jc, 0:T_SIZE], pv[:, 0:T_SIZE],
                        bcast_g[:, pair, 0:T_SIZE], mult)
                    nc.vector.tensor_tensor(
                        attnoutT[:, jc, 2 * T_SIZE:N], pv[:, 2 * T_SIZE:N],
                        bcast_g[:, pair, 2 * T_SIZE:N], mult)
                    nc.vector.tensor_tensor(
                        attnoutT[:, jc, T_SIZE:2 * T_SIZE], pvit[:],
                        bcast_g[:, pair, N:NW], mult)
                    if pair == 0:
                        fill(1)

        # prologue: batch 0's qk/v chains run first
        for emit in qkv_chains(0):
            emit()
        for b in range(n_batches):
            fillers = deque()
            if b + 1 < n_batches:
                fillers.extend(qkv_chains(b + 1))
            if b >= 1:
                fillers.extend(proj_chains(b - 1))
            emit_attention(b, fillers)
            while fillers:
                fillers.popleft()()
        for emit in proj_chains(n_batches - 1):
            emit()

    nc.compile()
    return nc


_CACHED_NC = None


def _get_nc():
    global _CACHED_NC
    if _CACHED_NC is None:
        _CACHED_NC = build(BPC)
    return _CACHED_NC


def kernel(x, w_qkv, w_proj, b_proj):
    from concourse.bass_utils import run_bass_kernel_spmd

    nc = _get_nc()

    wqkv_bf = np.ascontiguousarray(w_qkv.astype(BF16))
    wproj_bf = np.ascontiguousarray(w_proj.astype(BF16))
    bproj_f = np.ascontiguousarray(b_proj.astype(np.float32))

    in_maps = []
    for c in range(NCORES):
        xc = x[c * BPC:(c + 1) * BPC].reshape(BPC * N, DIM)
        xT = np.ascontiguousarray(xc.T.astype(BF16))
        in_maps.append({
            "xT": xT,
            "wqkv": wqkv_bf,
            "wproj": wproj_bf,
            "bproj": bproj_f,
        })

    res = run_bass_kernel_spmd(nc, in_maps, core_ids=list(range(NCORES)))
    outs = [
        np.ascontiguousarray(res.results[c]["outT"].T).reshape(BPC, N, DIM)
        for c in range(NCORES)
    ]
    return np.concatenate(outs, axis=0)


if __name__ == "__main__":
    rng = np.random.default_rng(0)
    x = rng.standard_normal((B, N, DIM), dtype=np.float32)
    w_qkv = (rng.standard_normal((DIM, 3 * DIM), dtype=np.float32) * 0.02)
    w_proj = (rng.standard_normal((DIM, DIM), dtype=np.float32) * 0.02)
    b_proj = np.zeros((DIM,), dtype=np.float32)
    out = kernel(x, w_qkv, w_proj, b_proj)
    print("out", out.shape, out.dtype, float(np.abs(out).max()))


# revision 33
# speedup vs baseline: 1.1120x; 1.0207x over previous
"""Trainium2 Bass kernel for nn_Attention_8924942041930 (sparse_attention).

Reference computation (per batch of N=384 tokens = [t(64) | it(64) | s(256)]):
  qkv = x @ w_qkv
  mix attention: (t+s) queries over all N keys
  inherent attention: it queries over it keys only
  out = concat(t, it, s) @ w_proj + b_proj

Sharding: data-parallel over batch, 8 batches per NeuronCore (64 total / 8).

Per-core dataflow (all matmuls bf16 with f32 PSUM accumulation):
  - host supplies x^T [768, 3072] bf16 so the f-contraction lands on SBUF
    partitions with no on-chip transposes
  - qk^T = w_qkv^T @ x^T per batch -> [hd, token] per head (head pair per
    128-partition chunk: even head rows 0-63, odd head rows 64-127)
  - v = x @ w_v -> [token, hd] per head (PV lhsT needs token on partitions)
  - S^T[k, q] = k^T.T @ q^T per (head, k-chunk); K=64 so head pairs are
    row-packed in the PE array via tile_position rows 0/64 (MMs emitted
    A/B-interleaved so they run concurrently)
  - attn^T = exp(S^T * scale) on ScalarE (logits are tiny: no max-subtract)
  - ts^T = v.T @ attn^T col-packed per head pair (tile_position cols 0/64)
    so the pair lands stacked [128, q] = the proj rhs chunk layout
  - denominators = ones.T @ attn^T as M=1 matmuls col-tiled 4 heads/bank
    (emitted 4-head-interleaved for concurrency); inherent sums share the
    same bank at cols N:N+64
  - normalize: fast reciprocal + DRAM-bounce partition-broadcast + DVE mult
  - inherent attention reuses S^T chunk 0 rows/cols 64:128 (it x it block)
  - out^T = w_proj^T @ attnout^T + b_proj -> host transposes back

Scheduling: the PE instruction queue is strict FIFO, and the attention
matmuls are gated by ScalarE exp throughput, so qk/v chains of batch b+1
and proj chains of batch b-1 are emitted interleaved between the attention
groups of batch b as PE filler work.
"""

import sys

import numpy as np

if "/opt/trn_rl_repo" not in sys.path:
    sys.path.insert(0, "/opt/trn_rl_repo")

import ml_dtypes

B = 64
N = 384
DIM = 768
H = 12
HD = 64
T_SIZE = 64
S_SIZE = 256
SCALE = HD ** -0.5
NCORES = 8
BPC = B // NCORES  # batches per core
FCH = DIM // 128  # feature chunks of 128
P = 128
NW = N + T_SIZE  # sums/bcast width: mix cols 0:N, inherent cols N:N+64

BF16 = ml_dtypes.bfloat16


def build(n_batches=BPC, sim_safe=False):
    """Build the per-core Bass program (SPMD: same NEFF, per-core data)."""
    import concourse.mybir as mybir
    import concourse.tile as tile
    from concourse import bacc
    from collections import deque
    from contextlib import ExitStack

    bf16 = mybir.dt.bfloat16
    f32 = mybir.dt.float32
    Exp = mybir.ActivationFunctionType.Exp
    mult = mybir.AluOpType.mult
    ntok = n_batches * N

    nc = bacc.Bacc("TRN2", target_bir_lowering=False, debug=False,
                   num_devices=NCORES)
    xT = nc.dram_tensor("xT", [DIM, ntok], bf16, kind="ExternalInput")
    wqkv = nc.dram_tensor("wqkv", [DIM, 3 * DIM], bf16, kind="ExternalInput")
    wproj = nc.dram_tensor("wproj", [DIM, DIM], bf16, kind="ExternalInput")
    bproj = nc.dram_tensor("bproj", [DIM], f32, kind="ExternalInput")
    outT = nc.dram_tensor("outT", [DIM, ntok], f32, kind="ExternalOutput")

    xT_r = xT.rearrange("(o p) t -> p o t", p=P)
    wqkv_r = wqkv.rearrange("(o p) c -> p o c", p=P)
    wproj_r = wproj.rearrange("(o p) c -> p o c", p=P)
    bproj_r = bproj.rearrange("(o p) -> p o", p=P)
    outT_r = outT.rearrange("(o p) t -> p o t", p=P)

    with tile.TileContext(nc) as tc, ExitStack() as ctx:
        const = ctx.enter_context(tc.tile_pool(name="const", bufs=1))
        qk_pool = ctx.enter_context(tc.tile_pool(name="qk", bufs=2))
        v_pool = ctx.enter_context(tc.tile_pool(name="v", bufs=2))
        attn_pool = ctx.enter_context(tc.tile_pool(name="attn", bufs=8))
        ao_pool = ctx.enter_context(tc.tile_pool(name="ao", bufs=2))
        rec_pool = ctx.enter_context(tc.tile_pool(name="rec", bufs=3))
        bc_pool = ctx.enter_context(tc.tile_pool(name="bc", bufs=3))
        out_pool = ctx.enter_context(tc.tile_pool(name="outp", bufs=2))
        dram_pool = ctx.enter_context(tc.tile_pool(name="dramp", bufs=3,
                                                   space="DRAM"))
        # PSUM budget: one fully fungible pool, all 8 banks shared
        work_ps = ctx.enter_context(tc.tile_pool(name="workps", bufs=8,
                                                 space="PSUM"))
        pv_ps = work_ps

        # --- persistent tensors (per-chunk tiles => finer DMA deps) ---
        xT_ch = [const.tile([P, ntok], bf16, tag=f"xT{o}", name=f"xT{o}")
                 for o in range(FCH)]
        wqkv_ch = [const.tile([P, 3 * DIM], bf16, tag=f"wqkv{o}",
                              name=f"wqkv{o}") for o in range(FCH)]
        wproj_ch = [const.tile([P, DIM], bf16, tag=f"wproj{o}",
                               name=f"wproj{o}") for o in range(FCH)]
        # phase 1: slices the first chains touch, alternating queues
        for o in range(FCH):
            eng = nc.sync if o % 2 == 0 else nc.gpsimd
            eng.dma_start(wqkv_ch[o][:, 0:N], wqkv_r[:, o, 0:N])
            eng.dma_start(xT_ch[o][:, 0:N], xT_r[:, o, 0:N])
        # phase 2: rest of wqkv (later qk chunks + v weights)
        for o in range(FCH):
            eng = nc.sync if o % 2 == 0 else nc.gpsimd
            eng.dma_start(wqkv_ch[o][:, N:2 * DIM], wqkv_r[:, o, N:2 * DIM])
            eng.dma_start(wqkv_ch[o][:, 2 * DIM:3 * DIM],
                          wqkv_r[:, o, 2 * DIM:3 * DIM])
        # phase 3: remaining activations and proj weights
        q = (ntok - N) // 3 if n_batches > 1 else 0
        for piece in range(3 if q else 0):
            for o in range(FCH):
                eng = nc.sync if o % 2 == 0 else nc.gpsimd
                eng.dma_start(xT_ch[o][:, N + piece * q:N + (piece + 1) * q],
                              xT_r[:, o, N + piece * q:N + (piece + 1) * q])
        for o in range(FCH):
            nc.sync.dma_start(wproj_ch[o][:], wproj_r[:, o, :])
        bproj_sb = const.tile([P, FCH], f32, tag="bproj")
        nc.sync.dma_start(bproj_sb[:], bproj_r[:])
        ones_sb = const.tile([P, 1], bf16, tag="ones")
        nc.gpsimd.memset(ones_sb[:], 1.0)

        qkTs = {}   # b -> qkT tile
        v_sbs = {}  # b -> v tile
        aos = {}    # b -> attnoutT tile

        def qkv_chains(b):
            """Yield one emitter per accumulation chain (18 per batch)."""
            t0 = b * N
            qkT = qk_pool.tile([P, 2 * FCH, N], bf16, tag="qkT")
            qkTs[b] = qkT

            def qk_chain(cc):
                def emit():
                    ps = work_ps.tile([P, N], mybir.dt.float32, tag="w")
                    for fo in range(FCH):
                        nc.tensor.matmul(
                            ps[:],
                            wqkv_ch[fo][:, cc * P:(cc + 1) * P],
                            xT_ch[fo][:, t0:t0 + N],
                            start=(fo == 0), stop=(fo == FCH - 1),
                        )
                    if cc % 3 == 2:
                        nc.scalar.copy(qkT[:, cc, :], ps[:])
                    else:
                        nc.vector.tensor_copy(qkT[:, cc, :], ps[:])
                return emit

            for cc in range(2 * FCH):
                yield qk_chain(cc)

            v_sb = v_pool.tile([P, 3, H, HD], bf16, tag="v")
            v_sbs[b] = v_sb

            def v_chain(tch, half):
                def emit():
                    ps = work_ps.tile([P, N], mybir.dt.float32, tag="w")
                    for fo in range(FCH):
                        nc.tensor.matmul(
                            ps[:],
                            xT_ch[fo][:, t0 + tch * P:t0 + (tch + 1) * P],
                            wqkv_ch[fo][:,
                                         2 * DIM + half * N:
                                         2 * DIM + (half + 1) * N],
                            start=(fo == 0), stop=(fo == FCH - 1),
                        )
                    nc.vector.tensor_copy(
                        v_sb[:, tch, half * 6:(half + 1) * 6, :],
                        ps[:].rearrange("p (h d) -> p h d", d=HD),
                    )
                return emit

            for tch in range(3):
                for half in range(2):
                    yield v_chain(tch, half)

        def proj_chains(b):
            """Yield one emitter per proj output chunk (6) + the store."""
            t0 = b * N
            ao = aos.pop(b)
            outstage = out_pool.tile([P, FCH, N], mybir.dt.float32,
                                     tag="outs")

            def proj_chain(cc):
                def emit():
                    ps = work_ps.tile([P, N], mybir.dt.float32, tag="w")
                    for fo in range(FCH):
                        nc.tensor.matmul(
                            ps[:],
                            wproj_ch[fo][:, cc * P:(cc + 1) * P],
                            ao[:, fo, :],
                            start=(fo == 0), stop=(fo == FCH - 1),
                        )
                    nc.vector.tensor_scalar_add(outstage[:, cc, :], ps[:],
                                                bproj_sb[:, cc:cc + 1])
                    nc.sync.dma_start(outT_r[:, cc, t0:t0 + N],
                                      outstage[:, cc, :])
                return emit

            for cc in range(FCH):
                yield proj_chain(cc)

        def emit_attention(b, fillers):
            """Attention for batch b; pops PE filler chains at stall points."""
            qkT = qkTs.pop(b)
            v_sb = v_sbs[b]

            def fill(k):
                for _ in range(k):
                    if fillers:
                        fillers.popleft()()

            attnoutT = ao_pool.tile([P, FCH, N], bf16, tag="aoT")
            aos[b] = attnoutT

            for g in range(3):  # head groups of 4 (two pairs)
                heads = [4 * g + i for i in range(4)]
                attnTs = {}
                for h in heads:
                    attnTs[h] = attn_pool.tile([P, 3, N], bf16, tag="attnT",
                                               name=f"attnT_{h}")
                # S^T + exp: pairs row-packed; A/B interleaved per kc so the
                # two K=64 matmuls run concurrently in the array
                for pair in range(2):
                    hA, hB = heads[2 * pair], heads[2 * pair + 1]
                    j = hA // 2
                    for kc in range(3):
                        sA = work_ps.tile([P, N], mybir.dt.float32, tag="w", name="sA")
                        sB = work_ps.tile([P, N], mybir.dt.float32, tag="w", name="sB")
                        nc.tensor.matmul(
                            sA[:], qkT[0:64, FCH + j, kc * P:(kc + 1) * P],
                            qkT[0:64, j, :], start=True, stop=True,
                            tile_position=(0, 0))
                        nc.tensor.matmul(
                            sB[:], qkT[64:128, FCH + j, kc * P:(kc + 1) * P],
                            qkT[64:128, j, :], start=True, stop=True,
                            tile_position=(64, 0))
                        nc.scalar.activation(attnTs[hA][:, kc, :], sA[:],
                                             Exp, scale=SCALE)
                        nc.scalar.activation(attnTs[hB][:, kc, :], sB[:],
                                             Exp, scale=SCALE)
                        # cover the exp-paced s-slot recycle with dense work
                        fill(1)

                # denominators: 4 heads col-tiled; idx-inner emission so the
                # four M=1 matmuls stream concurrently
                sums = work_ps.tile([P, NW], mybir.dt.float32, tag="w",
                                    name="sums")
                if sim_safe:
                    nc.vector.memset(sums[:], 1.0)
                for kc in range(3):
                    for idx, h in enumerate(heads):
                        nc.tensor.matmul(
                            sums[32 * idx:32 * idx + 1, 0:N],
                            ones_sb[:, 0:1], attnTs[h][:, kc, :],
                            start=(kc == 0), stop=(kc == 2),
                            tile_position=(0, 32 * idx),
                            skip_group_check=(idx > 0),
                        )
                for idx, h in enumerate(heads):
                    nc.tensor.matmul(
                        sums[32 * idx:32 * idx + 1, N:NW],
                        ones_sb[64:128, 0:1], attnTs[h][64:128, 0, 64:128],
                        start=True, stop=True,
                        tile_position=(64, 32 * idx),
                        skip_group_check=(idx > 0),
                    )
                recips = rec_pool.tile([P, NW], mybir.dt.float32, tag="rec")
                nc.vector.reciprocal_approx_fast(recips[0:97, :],
                                                 sums[0:97, :])
                rec_dram = dram_pool.tile([4, NW], mybir.dt.float32, tag="rd")
                nc.gpsimd.dma_start(
                    rec_dram[:],
                    recips[:].rearrange("(a b) n -> a b n", b=32)[:, 0, :],
                )
                # broadcast: [128, pair, NW]; rows 0-63 = even (A) recips,
                # rows 64-127 = odd (B) recips
                bcast_g = bc_pool.tile([P, 2, NW], mybir.dt.float32,
                                       tag="bc")
                rec_pairs = rec_dram[:].rearrange("(p two) n -> p two n",
                                                  two=2)
                nc.gpsimd.dma_start(
                    bcast_g[0:64, :, :],
                    rec_pairs[:, 0, :].partition_broadcast(64))
                nc.gpsimd.dma_start(
                    bcast_g[64:128, :, :],
                    rec_pairs[:, 1, :].partition_broadcast(64))
                fill(1)

                for pair in range(2):
                    hA, hB = heads[2 * pair], heads[2 * pair + 1]
                    jc = hA // 2
                    attnA, attnB = attnTs[hA], attnTs[hB]

                    # PV mix: pair col-packed -> [128, N] stacked ts^T
                    pv = pv_ps.tile([P, N], mybir.dt.float32, tag="w", name="pv")
                    for kc in range(3):
                        nc.tensor.matmul(
                            pv[0:64, :], v_sb[:, kc, hA, :], attnA[:, kc, :],
                            start=(kc == 0), stop=(kc == 2),
                            tile_position=(0, 0),
                        )
                        nc.tensor.matmul(
                            pv[64:128, :], v_sb[:, kc, hB, :],
                            attnB[:, kc, :],
                            start=(kc == 0), stop=(kc == 2),
                            tile_position=(0, 64),
                            skip_group_check=True,
                        )
                    # PV inherent: it queries over it keys (chunk 0 64:128)
                    pvit = pv_ps.tile([P, T_SIZE], mybir.dt.float32,
                                      tag="w", name="pvit")
                    nc.tensor.matmul(
                        pvit[0:64, :], v_sb[64:128, 0, hA, :],
                        attnA[64:128, 0, 64:128],
                        start=True, stop=True, tile_position=(64, 0),
                    )
                    nc.tensor.matmul(
                        pvit[64:128, :], v_sb[64:128, 0, hB, :],
                        attnB[64:128, 0, 64:128],
                        start=True, stop=True, tile_position=(64, 64),
                        skip_group_check=True,
                    )
                    nc.vector.tensor_tensor(
                        attnoutT[:, jc, 0:T_SIZE], pv[:, 0:T_SIZE],
                        bcast_g[:, pair, 0:T_SIZE], mult)
                    nc.vector.tensor_tensor(
                        attnoutT[:, jc, 2 * T_SIZE:N], pv[:, 2 * T_SIZE:N],
                        bcast_g[:, pair, 2 * T_SIZE:N], mult)
                    nc.vector.tensor_tensor(
                        attnoutT[:, jc, T_SIZE:2 * T_SIZE], pvit[:],
                        bcast_g[:, pair, N:NW], mult)
                    if pair == 0:
                        fill(1)

        # prologue: batch 0's qk/v chains run first
        for emit in qkv_chains(0):
            emit()
        for b in range(n_batches):
            fillers = deque()
            if b + 1 < n_batches:
                fillers.extend(qkv_chains(b + 1))
            if b >= 1:
                fillers.extend(proj_chains(b - 1))
            emit_attention(b, fillers)
            while fillers:
                fillers.popleft()()
        for emit in proj_chains(n_batches - 1):
            emit()

    nc.compile()
    return nc


_CACHED_NC = None


def _get_nc():
    global _CACHED_NC
    if _CACHED_NC is None:
        _CACHED_NC = build(BPC)
    return _CACHED_NC


def kernel(x, w_qkv, w_proj, b_proj):
    from concourse.bass_utils import run_bass_kernel_spmd

    nc = _get_nc()

    wqkv_bf = np.ascontiguousarray(w_qkv.astype(BF16))
    wproj_bf = np.ascontiguousarray(w_proj.astype(BF16))
    bproj_f = np.ascontiguousarray(b_proj.astype(np.float32))

    in_maps = []
    for c in range(NCORES):
        xc = x[c * BPC:(c + 1) * BPC].reshape(BPC * N, DIM)
        xT = np.ascontiguousarray(xc.T.astype(BF16))
        in_maps.append({
            "xT": xT,
            "wqkv": wqkv_bf,
            "wproj": wproj_bf,
            "bproj": bproj_f,
        })

    res = run_bass_kernel_spmd(nc, in_maps, core_ids=list(range(NCORES)))
    outs = [
        np.ascontiguousarray(res.results[c]["outT"].T).reshape(BPC, N, DIM)
        for c in range(NCORES)
    ]
    return np.concatenate(outs, axis=0)


if __name__ == "__main__":
    rng = np.random.default_rng(0)
    x = rng.standard_normal((B, N, DIM), dtype=np.float32)
    w_qkv = (rng.standard_normal((DIM, 3 * DIM), dtype=np.float32) * 0.02)
    w_proj = (rng.standard_normal((DIM, DIM), dtype=np.float32) * 0.02)
    b_proj = np.zeros((DIM,), dtype=np.float32)
    out = kernel(x, w_qkv, w_proj, b_proj)
    print("out", out.shape, out.dtype, float(np.abs(out).max()))
